# revision 14
# baseline (speedup 1.0000x reference)
"""CRF forward-algorithm kernel for Trainium2 (8 NeuronCores, data-parallel over batch).

Math: the reference computes, per sample b,
    fv_{t+1}[next] = x_t[next] + logsumexp_prev(transit[next, prev] + fv_t[prev])   (t < len_b)
    alpha[b] = logsumexp_i(fv_{len_b}[i] + transit[STOP, i])

In linear space with E = exp(transit) this is
    w_{t+1} = exp(x_t) * (E @ w_t),      fv_t = log(w_t) + c_t
so each timestep is one 128x128 @ 128x32 matmul (PE) plus one elementwise
multiply (DVE).  We pre-scale exp(x) by 1/128 and renormalize every K=16 steps
(dividing each sample column by its colsum, recorded via the exact f32
reciprocal the device used) to stay in f32 range.  Because
alpha needs (E @ w_len)[STOP] and STOP = 127 is the last row of E, the per-step
capture is just row 127 of the state, DMA'd out once per 16 steps.  The final
log/gather bookkeeping (O(B*T) scalar work) runs on host in float64.
"""

import sys

sys.path.insert(0, "/opt/trn_rl_repo")

import numpy as np
from contextlib import ExitStack

import concourse.bass as bass
import concourse.tile as tile
import concourse.mybir as mybir
from concourse import bacc, bass_utils

# Problem constants (hardcoded per contract).
B, T, L = 256, 512, 128
NCORES = 8
BC = B // NCORES          # 32 samples per core
K = 16                    # renormalize / capture period (divides T)
CH = 128                  # x chunk length in timesteps (divides T, multiple of K)
NBLK = T // K             # 32
LN128 = float(np.log(128.0))
F32 = mybir.dt.float32

_CACHED_NC = None


def _build_bass():
    """Build the single-core Bass program (shared SPMD across 8 cores)."""
    nc = bacc.Bacc("TRN2", debug=False)

    xT = nc.dram_tensor("xT", [L, T * BC], F32, kind="ExternalInput").ap()
    trT = nc.dram_tensor("trT", [L, L], F32, kind="ExternalInput").ap()
    # hist: 32 blocks of [16 slots x 32 samples] of w[127], then 32 entries of
    # (E @ w_512)[127].
    hist = nc.dram_tensor("hist", [1, T * BC + BC], F32, kind="ExternalOutput").ap()
    rhist = nc.dram_tensor("rhist", [1, NBLK * BC], F32, kind="ExternalOutput").ap()

    with tile.TileContext(nc) as tc, ExitStack() as ctx:
        const_pool = ctx.enter_context(tc.tile_pool(name="const", bufs=1))
        state_pool = ctx.enter_context(tc.tile_pool(name="state", bufs=1))
        xin_pool = ctx.enter_context(tc.tile_pool(name="xin", bufs=2))
        ex_pool = ctx.enter_context(tc.tile_pool(name="ex", bufs=3))
        ps_pool = ctx.enter_context(tc.tile_pool(name="ps", bufs=4, space="PSUM"))
        ps2_pool = ctx.enter_context(tc.tile_pool(name="ps2", bufs=2, space="PSUM"))

        # Constants: E_sb[prev, next] = exp(transit[next, prev]) is the matmul
        # lhsT; ones vectors drive the colsum / broadcast matmuls.
        nbias = const_pool.tile([L, 1], F32)
        nc.vector.memset(nbias[:], -LN128)
        zbias = const_pool.tile([L, 1], F32)
        nc.vector.memset(zbias[:], 0.0)
        tr_sb = const_pool.tile([L, L], F32)
        nc.sync.dma_start(tr_sb[:], trT[:, :])
        E_sb = const_pool.tile([L, L], F32)
        nc.scalar.activation(E_sb[:], tr_sb[:], mybir.ActivationFunctionType.Exp,
                             bias=zbias[:])
        ones_col = const_pool.tile([L, 1], F32)
        nc.vector.memset(ones_col[:], 1.0)
        ones_row = const_pool.tile([1, L], F32)
        nc.vector.memset(ones_row[:], 1.0)

        # Reciprocal history (normalizers), written in place by the reciprocal.
        rh_sb = state_pool.tile([1, NBLK * BC], F32)

        # State ring: W[:, s*BC:(s+1)*BC] holds w at slot s (16 slots).
        W = state_pool.tile([L, K * BC], F32)
        nc.vector.memset(W[:, 0:BC], 0.0)
        nc.vector.memset(W[0:1, 0:BC], 1.0)  # w_0 = onehot(START=0)

        for c in range(T // CH):
            xt = xin_pool.tile([L, CH * BC], F32)
            nc.sync.dma_start(xt[:], xT[:, c * CH * BC:(c + 1) * CH * BC])
            for jj in range(CH // K):
                j = c * (CH // K) + jj  # global 16-step block index
                ex = ex_pool.tile([L, K * BC], F32)
                nc.scalar.activation(
                    ex[:], xt[:, jj * K * BC:(jj + 1) * K * BC],
                    mybir.ActivationFunctionType.Exp, bias=nbias[:],
                )
                for i in range(K):
                    t = j * K + i
                    src = W[:, (t % K) * BC:((t % K) + 1) * BC]
                    ds = (t + 1) % K
                    dst = W[:, ds * BC:(ds + 1) * BC]
                    P = ps_pool.tile([L, BC], F32, tag="P")
                    nc.tensor.matmul(P[:], E_sb[:], src, start=True, stop=True)
                    nc.vector.tensor_mul(dst, ex[:, i * BC:(i + 1) * BC], P[:])
                # Capture row 127 (pre-normalization) for this block.
                nc.sync.dma_start(hist[0:1, j * K * BC:(j + 1) * K * BC],
                                  W[127:128, :])
                # Renormalize: divide each column by its colsum; record 1/s.
                s = ps2_pool.tile([1, BC], F32, tag="s")
                nc.tensor.matmul(s[:], ones_col[:], W[:, 0:BC], start=True,
                                 stop=True)
                r = rh_sb[0:1, j * BC:(j + 1) * BC]
                nc.vector.reciprocal(r, s[:])
                R = ps2_pool.tile([L, BC], F32, tag="R")
                nc.tensor.matmul(R[:], ones_row[:], r, start=True, stop=True)
                nc.vector.tensor_mul(W[:, 0:BC], W[:, 0:BC], R[:])

        # Final (E @ w_512)[127] for samples with len == T.
        P = ps_pool.tile([L, BC], F32, tag="P")
        nc.tensor.matmul(P[:], E_sb[:], W[:, 0:BC], start=True, stop=True)
        cap = state_pool.tile([L, BC], F32)
        nc.vector.tensor_copy(cap[:], P[:])
        nc.sync.dma_start(hist[0:1, T * BC:T * BC + BC], cap[127:128, :])
        nc.sync.dma_start(rhist[0:1, :], rh_sb[:])

    nc.compile()
    return nc


def _get_nc():
    global _CACHED_NC
    if _CACHED_NC is None:
        _CACHED_NC = _build_bass()
    return _CACHED_NC


def run_on_device(x, transit_matrix, **spmd_kwargs):
    """Shard inputs, run the SPMD kernel on 8 cores, return per-core results."""
    xT = np.ascontiguousarray(np.asarray(x, np.float32).transpose(2, 1, 0))
    trT = np.ascontiguousarray(np.asarray(transit_matrix, np.float32).T)
    in_maps = []
    for c in range(NCORES):
        xc = np.ascontiguousarray(xT[:, :, c * BC:(c + 1) * BC]).reshape(L, T * BC)
        in_maps.append({"xT": xc, "trT": trT})
    nc = _get_nc()
    return bass_utils.run_bass_kernel_spmd(
        nc, in_maps, core_ids=list(range(NCORES)), **spmd_kwargs
    )


def finish_on_host(results, x, lengths):
    """Reconstruct alpha[b] in float64 from the device captures."""
    lengths = np.asarray(lengths).astype(np.int64)
    x = np.asarray(x)
    alpha = np.empty(B, np.float64)
    for c in range(NCORES):
        hist = results[c]["hist"].reshape(-1).astype(np.float64)
        rh = results[c]["rhist"].reshape(-1).astype(np.float64)
        lnS = -np.log(rh.reshape(NBLK, BC))          # ln s_j per norm j
        cum = np.zeros((NBLK + 1, BC))
        cum[1:] = np.cumsum(lnS, axis=0)
        hist_blk = hist[:T * BC].reshape(NBLK, K, BC)  # [block, slot, sample]
        cap512 = hist[T * BC:]

        ln = lengths[c * BC:(c + 1) * BC]            # (BC,)
        bi = np.arange(BC)
        full = ln == T
        nf = ~full
        out = np.empty(BC, np.float64)
        # len == T: alpha = T*ln128 + sum(all lnS) + ln((E @ w_T)[127])
        out[full] = T * LN128 + cum[NBLK, bi[full]] + np.log(cap512[full])
        # len < T: capture w_{len+1}[127] = exp(x[b,len,127])/128 * (E@w_len)[127]
        tt = ln[nf] + 1
        blk = tt // K
        slot = tt % K
        wrap = slot == 0
        blk = np.where(wrap, blk - 1, blk)
        cap = hist_blk[blk, slot, bi[nf]]
        x127 = x[c * BC + bi[nf], ln[nf], 127].astype(np.float64)
        out[nf] = (ln[nf] * LN128 + cum[ln[nf] // K, bi[nf]]
                   + np.log(cap) - x127 + LN128)
        alpha[c * BC:(c + 1) * BC] = out
    return alpha.astype(np.float32)


def kernel(x, transit_matrix, lengths):
    x = np.asarray(x, np.float32)
    assert x.shape == (B, T, L), x.shape
    res = run_on_device(x, transit_matrix)
    return finish_on_host(res.results, x, lengths)


# revision 20
# speedup vs baseline: 1.9937x; 1.9937x over previous
"""CRF forward-algorithm kernel for Trainium2 (8 NeuronCores, data-parallel over batch).

Math: the reference computes, per sample b,
    fv_{t+1}[next] = x_t[next] + logsumexp_prev(transit[next, prev] + fv_t[prev])   (t < len_b)
    alpha[b] = logsumexp_i(fv_{len_b}[i] + transit[STOP, i])

In linear space with E = exp(transit) this is
    w_{t+1} = exp(x_t) * (E @ w_t),      fv_t = log(w_t) + c_t
so each timestep is one 128x128 @ 128x32 matmul (PE, fp16 weights cached-ish via
FWL) plus one elementwise multiply (DVE).  exp(x) is pre-scaled by 1/256 and the
state is renormalized every K=4 steps to stay in fp16 range: the normalizer is
row 0 of the already-computed matmul output P (any per-column scale works), its
fp16 reciprocal r is recorded for the host, broadcast via a K=1 matmul, and the
division is folded into the exp(x) slice of step tau+2 so the renorm never sits
on the serial matmul->multiply->matmul chain.  Because alpha needs
(E @ w_len)[STOP] and STOP = 127 is the last row of E, the per-step capture is
just row 127 of the state; the state lives in two alternating 16-slot rings so
row 127 of a finished ring is copied (ACT) and DMA'd out with no WAR stall.
The batch is split into G=2 interleaved groups so the PE matmul of one group
overlaps the DVE multiply of the other.  The final log/gather bookkeeping
(O(B*T) scalar work) runs on host in float64 from the captures.
"""

import sys

sys.path.insert(0, "/opt/trn_rl_repo")

import numpy as np
from contextlib import ExitStack

import concourse.bass as bass
import concourse.tile as tile
import concourse.mybir as mybir
from concourse import bacc, bass_utils

# Problem constants (hardcoded per contract).
B, T, L = 256, 512, 128
NCORES = 8
BC = B // NCORES          # 32 samples per core
K = 4                     # renormalization period
D = 2                     # renorm application delay (steps after tau)
CAPB = 16                 # capture block (ring size)
CH = 128                  # x chunk length in timesteps
NCAP = T // CAPB          # 32 capture blocks
NNORM = 127               # norms m=0..126: tau=4(m+1)<=508, applied at tau+D<=510
G = 2                     # interleaved batch groups
GS = BC // G
SCALE_LN = float(np.log(256.0))
F32 = mybir.dt.float32
DT = mybir.dt.float16     # state/weights dtype

_CACHED_NC = None


def _build_bass():
    """Build the single-core Bass program (shared SPMD across 8 cores)."""
    nc = bacc.Bacc("TRN2", debug=False)

    xT = nc.dram_tensor("xT", [L, T * BC], F32, kind="ExternalInput").ap()
    trT = nc.dram_tensor("trT", [L, L], F32, kind="ExternalInput").ap()
    # hist[j*CAPB*BC + s*BC + b] = w_{16j+1+s}[127, b]; tail BC entries are
    # (E @ w_512)[127].
    hist = nc.dram_tensor("hist", [1, T * BC + BC], DT, kind="ExternalOutput").ap()
    rhist = nc.dram_tensor("rhist", [1, NNORM * BC], DT, kind="ExternalOutput").ap()

    with tile.TileContext(nc) as tc, ExitStack() as ctx, \
            nc.allow_low_precision(reason="fp16 state validated against f64 ref"):
        const_pool = ctx.enter_context(tc.tile_pool(name="const", bufs=1))
        state_pool = ctx.enter_context(tc.tile_pool(name="state", bufs=1))
        xin_pool = ctx.enter_context(tc.tile_pool(name="xin", bufs=2))
        ex_pool = ctx.enter_context(tc.tile_pool(name="ex", bufs=3))
        ps_pool = ctx.enter_context(tc.tile_pool(name="ps", bufs=3, space="PSUM"))
        ps2_pool = ctx.enter_context(tc.tile_pool(name="ps2", bufs=1, space="PSUM"))

        # Constants.
        nbias = const_pool.tile([L, 1], F32)
        nc.vector.memset(nbias[:], -SCALE_LN)
        zbias = const_pool.tile([L, 1], F32)
        nc.vector.memset(zbias[:], 0.0)
        tr_sb = const_pool.tile([L, L], F32)
        nc.sync.dma_start(tr_sb[:], trT[:, :])
        E_sb = const_pool.tile([L, L], DT)
        nc.scalar.activation(E_sb[:], tr_sb[:], mybir.ActivationFunctionType.Exp,
                             bias=zbias[:])
        ones_row = const_pool.tile([1, L], DT)
        nc.vector.memset(ones_row[:], 1.0)

        # Reciprocal history (one fp16 reciprocal per norm per sample).
        rh_sb = state_pool.tile([1, NNORM * BC], DT)

        # Two state rings: ring(j) = j%2 holds w_{16j+1..16j+16} in slots 0..15.
        WA = state_pool.tile([L, CAPB * BC], DT)
        WB = state_pool.tile([L, CAPB * BC], DT)
        rings = [WA, WB]
        # w_0 = onehot(START=0) lives at ring 1, slot 15.
        nc.vector.memset(WB[:, 15 * BC:16 * BC], 0.0)
        nc.vector.memset(WB[0:1, 15 * BC:16 * BC], 1.0)

        def wslot(t):
            """AP of w_t (full BC columns)."""
            ring = rings[((t - 1) // CAPB) % 2]
            s = (t - 1) % CAPB
            return ring[:, s * BC:(s + 1) * BC]

        ex_tiles = {}   # granule index -> ex tile (CAPB steps each)
        pend_R = None   # (R psum tile, application step)

        for c in range(T // CH):
            xt = xin_pool.tile([L, CH * BC], F32)
            nc.sync.dma_start(xt[:], xT[:, c * CH * BC:(c + 1) * CH * BC])
            for jj in range(CH // CAPB):
                j = c * (CH // CAPB) + jj   # capture block index
                ex = ex_pool.tile([L, CAPB * BC], DT)
                nc.scalar.activation(
                    ex[:], xt[:, jj * CAPB * BC:(jj + 1) * CAPB * BC],
                    mybir.ActivationFunctionType.Exp, bias=nbias[:],
                )
                ex_tiles[j] = ex
                for i in range(CAPB):
                    t = j * CAPB + i
                    # Apply a pending renorm to this step's ex slice.
                    if pend_R is not None and pend_R[1] == t:
                        R = pend_R[0]
                        nc.vector.tensor_mul(ex[:, i * BC:(i + 1) * BC],
                                             ex[:, i * BC:(i + 1) * BC], R[:])
                        pend_R = None
                    src = wslot(t)
                    dst = wslot(t + 1)
                    Ps = []
                    for g in range(G):
                        P = ps_pool.tile([L, GS], F32, tag=f"P{g}")
                        nc.tensor.matmul(P[:], E_sb[:],
                                         src[:, g * GS:(g + 1) * GS],
                                         start=True, stop=True)
                        Ps.append(P)
                    for g in range(G):
                        nc.vector.tensor_mul(dst[:, g * GS:(g + 1) * GS],
                                             ex[:, i * BC + g * GS:
                                                i * BC + (g + 1) * GS],
                                             Ps[g][:])
                    # Renorm trigger: tau = t is a multiple of K (tau = 4(m+1));
                    # normalizer = P_tau[0, :], applied to ex of step tau+D.
                    if t % K == 0 and t > 0 and t + D <= T - 2:
                        m = t // K - 1
                        for g in range(G):
                            nc.vector.reciprocal(
                                rh_sb[0:1, m * BC + g * GS:m * BC + (g + 1) * GS],
                                Ps[g][0:1, :])
                        R = ps2_pool.tile([L, BC], F32, tag="R")
                        nc.tensor.matmul(R[:], ones_row[:],
                                         rh_sb[0:1, m * BC:(m + 1) * BC],
                                         start=True, stop=True)
                        pend_R = (R, t + D)
                # Capture row 127 of the finished ring (w_{16j+1..16j+16});
                # the double ring gives this DMA 16 steps of WAR slack.
                ring = rings[j % 2]
                nc.sync.dma_start(
                    hist[0:1, j * CAPB * BC:(j + 1) * CAPB * BC],
                    ring[127:128, :])
                if j - 2 in ex_tiles:
                    del ex_tiles[j - 2]

        # Final (E @ w_512)[127] for samples with len == T.
        Pf = ps_pool.tile([L, BC], F32, tag="P0")
        nc.tensor.matmul(Pf[:], E_sb[:], wslot(T), start=True, stop=True)
        capf = state_pool.tile([L, BC], DT)
        nc.vector.tensor_copy(capf[:], Pf[:])
        nc.sync.dma_start(hist[0:1, T * BC:T * BC + BC], capf[127:128, :])
        nc.sync.dma_start(rhist[0:1, :], rh_sb[:])

    nc.compile()
    return nc


def _get_nc():
    global _CACHED_NC
    if _CACHED_NC is None:
        _CACHED_NC = _build_bass()
    return _CACHED_NC


def run_on_device(x, transit_matrix, **spmd_kwargs):
    """Shard inputs, run the SPMD kernel on 8 cores, return per-core results."""
    xT = np.ascontiguousarray(np.asarray(x, np.float32).transpose(2, 1, 0))
    trT = np.ascontiguousarray(np.asarray(transit_matrix, np.float32).T)
    in_maps = []
    for c in range(NCORES):
        xc = np.ascontiguousarray(xT[:, :, c * BC:(c + 1) * BC]).reshape(L, T * BC)
        in_maps.append({"xT": xc, "trT": trT})
    nc = _get_nc()
    return bass_utils.run_bass_kernel_spmd(
        nc, in_maps, core_ids=list(range(NCORES)), **spmd_kwargs
    )


def finish_on_host(results, x, lengths):
    """Reconstruct alpha[b] in float64 from the device captures.

    fv_t = ln(w_t) + t*SCALE_LN + sum of ln(s_m) over norms applied before t
    (norm m: s_m = 1/r_m, r_m recorded; applied at step a_m = 4(m+1)+D).
    For len < T the capture is w_{len+1}[127] = exp(x[b,len,127])/256 *
    (E @ w_len)[127] (with the step-len renorm folded in when a_m == len), which
    collapses to the uniform formula below; for len == T the tail capture is
    (E @ w_512)[127] directly.
    """
    lengths = np.asarray(lengths).astype(np.int64)
    x = np.asarray(x)
    alpha = np.empty(B, np.float64)
    for c in range(NCORES):
        hist = results[c]["hist"].reshape(-1).astype(np.float64)
        rh = results[c]["rhist"].reshape(-1).astype(np.float64)
        lnS = -np.log(rh.reshape(NNORM, BC))          # ln s_m per norm m
        cum = np.zeros((NNORM + 1, BC))
        cum[1:] = np.cumsum(lnS, axis=0)
        hist_blk = hist[:T * BC].reshape(T, BC)       # hist_blk[t-1] = w_t[127]
        cap512 = hist[T * BC:]

        ln = lengths[c * BC:(c + 1) * BC]             # (BC,)
        bi = np.arange(BC)
        full = ln == T
        nf = ~full
        out = np.empty(BC, np.float64)
        out[full] = T * SCALE_LN + cum[NNORM, bi[full]] + np.log(cap512[full])
        cap = hist_blk[ln[nf], bi[nf]]                # w_{len+1}[127]
        x127 = x[c * BC + bi[nf], ln[nf], 127].astype(np.float64)
        # norms applied at a_m <= len: count = clip((len-2)//4, 0, NNORM)
        nidx = np.clip((ln[nf] - 2) // 4, 0, NNORM)
        out[nf] = (np.log(cap) - x127 + (ln[nf] + 1) * SCALE_LN
                   + cum[nidx, bi[nf]])
        alpha[c * BC:(c + 1) * BC] = out
    return alpha.astype(np.float32)


def kernel(x, transit_matrix, lengths):
    x = np.asarray(x, np.float32)
    assert x.shape == (B, T, L), x.shape
    res = run_on_device(x, transit_matrix)
    return finish_on_host(res.results, x, lengths)


# revision 27
# speedup vs baseline: 2.1016x; 1.0541x over previous
"""CRF forward-algorithm kernel for Trainium2 (8 NeuronCores, data-parallel over batch).

Math: the reference computes, per sample b,
    fv_{t+1}[next] = x_t[next] + logsumexp_prev(transit[next, prev] + fv_t[prev])   (t < len_b)
    alpha[b] = logsumexp_i(fv_{len_b}[i] + transit[STOP, i])

In linear space with E = exp(transit) this is
    w_{t+1} = exp(x_t) * (E @ w_t),      fv_t = log(w_t) + c_t
so each timestep is one 128x128 @ 128x32 matmul (PE, fp16 weights cached-ish via
FWL) plus one elementwise multiply (DVE).  exp(x) is pre-scaled by 1/256 and the
state is renormalized every K=4 steps to stay in fp16 range: the normalizer is
row 0 of the already-computed matmul output P (any per-column scale works), its
fp16 reciprocal r is recorded for the host, broadcast via a K=1 matmul, and the
division is folded into the exp(x) slice of step tau+2 so the renorm never sits
on the serial matmul->multiply->matmul chain.  Because alpha needs
(E @ w_len)[STOP] and STOP = 127 is the last row of E, the per-step capture is
just row 127 of the state; the state lives in two alternating 16-slot rings so
row 127 of a finished ring is copied (ACT) and DMA'd out with no WAR stall.
The batch is split into G=2 interleaved groups so the PE matmul of one group
overlaps the DVE multiply of the other.  The final log/gather bookkeeping
(O(B*T) scalar work) runs on host in float64 from the captures.
"""

import sys

sys.path.insert(0, "/opt/trn_rl_repo")

import numpy as np
from contextlib import ExitStack

import concourse.bass as bass
import concourse.tile as tile
import concourse.mybir as mybir
from concourse import bacc, bass_utils

# Problem constants (hardcoded per contract).
B, T, L = 256, 512, 128
NCORES = 8
BC = B // NCORES          # 32 samples per core
K = 8                     # renormalization period
D = 3                     # renorm application delay (steps after tau)
CAPB = 16                 # capture block (ring size)
CH = 128                  # x chunk length in timesteps
NCAP = T // CAPB          # 32 capture blocks
NNORM = 63                # norms m=0..62: tau=8(m+1)<=504, applied at tau+D
G = 2                     # interleaved batch groups
GS = BC // G
SCALE_LN = float(np.log(256.0))
F32 = mybir.dt.float32
DT = mybir.dt.float16     # state/weights dtype

_CACHED_NC = None


def _build_bass():
    """Build the single-core Bass program (shared SPMD across 8 cores)."""
    nc = bacc.Bacc("TRN2", debug=False)

    xT = nc.dram_tensor("xT", [L, T * BC], F32, kind="ExternalInput").ap()
    trT = nc.dram_tensor("trT", [L, L], F32, kind="ExternalInput").ap()
    # hist[j*CAPB*BC + s*BC + b] = w_{16j+1+s}[127, b]; tail BC entries are
    # (E @ w_512)[127].
    hist = nc.dram_tensor("hist", [1, T * BC + BC], DT, kind="ExternalOutput").ap()
    rhist = nc.dram_tensor("rhist", [1, NNORM * BC], DT, kind="ExternalOutput").ap()

    with tile.TileContext(nc) as tc, ExitStack() as ctx, \
            nc.allow_low_precision(reason="fp16 state validated against f64 ref"):
        const_pool = ctx.enter_context(tc.tile_pool(name="const", bufs=1))
        state_pool = ctx.enter_context(tc.tile_pool(name="state", bufs=1))
        xin_pool = ctx.enter_context(tc.tile_pool(name="xin", bufs=2))
        ex_pool = ctx.enter_context(tc.tile_pool(name="ex", bufs=3))
        ps_pool = ctx.enter_context(tc.tile_pool(name="ps", bufs=4, space="PSUM"))

        # Constants.
        nbias = const_pool.tile([L, 1], F32)
        nc.vector.memset(nbias[:], -SCALE_LN)
        zbias = const_pool.tile([L, 1], F32)
        nc.vector.memset(zbias[:], 0.0)
        tr_sb = const_pool.tile([L, L], F32)
        nc.sync.dma_start(tr_sb[:], trT[:, :])
        E_sb = const_pool.tile([L, L], DT)
        nc.scalar.activation(E_sb[:], tr_sb[:], mybir.ActivationFunctionType.Exp,
                             bias=zbias[:])

        # Reciprocal history (one fp16 reciprocal per norm per sample).
        rh_sb = state_pool.tile([1, NNORM * BC], DT)

        # Two state rings: ring(j) = j%2 holds w_{16j+1..16j+16} in slots 0..15.
        WA = state_pool.tile([L, CAPB * BC], DT)
        WB = state_pool.tile([L, CAPB * BC], DT)
        rings = [WA, WB]
        # w_0 = onehot(START=0) lives at ring 1, slot 15.
        nc.vector.memset(WB[:, 15 * BC:16 * BC], 0.0)
        nc.vector.memset(WB[0:1, 15 * BC:16 * BC], 1.0)

        def wslot(t):
            """AP of w_t (full BC columns)."""
            ring = rings[((t - 1) // CAPB) % 2]
            s = (t - 1) % CAPB
            return ring[:, s * BC:(s + 1) * BC]

        rbc_pool = ctx.enter_context(tc.tile_pool(name="rbc", bufs=2))

        ex_tiles = {}   # granule index -> ex tile (CAPB steps each)
        pend_R = None   # (broadcast reciprocal sbuf tile, application step)

        for c in range(T // CH):
            xt = xin_pool.tile([L, CH * BC], F32)
            nc.sync.dma_start(xt[:], xT[:, c * CH * BC:(c + 1) * CH * BC])
            for jj in range(CH // CAPB):
                j = c * (CH // CAPB) + jj   # capture block index
                ex = ex_pool.tile([L, CAPB * BC], DT)
                nc.scalar.activation(
                    ex[:], xt[:, jj * CAPB * BC:(jj + 1) * CAPB * BC],
                    mybir.ActivationFunctionType.Exp, bias=nbias[:],
                )
                ex_tiles[j] = ex
                for i in range(CAPB):
                    t = j * CAPB + i
                    # Apply a pending renorm to this step's ex slice.
                    if pend_R is not None and pend_R[1] == t:
                        R = pend_R[0]
                        nc.vector.tensor_mul(ex[:, i * BC:(i + 1) * BC],
                                             ex[:, i * BC:(i + 1) * BC], R[:])
                        pend_R = None
                    src = wslot(t)
                    dst = wslot(t + 1)
                    Ps = []
                    for g in range(G):
                        P = ps_pool.tile([L, GS], F32, tag=f"P{g}")
                        nc.tensor.matmul(P[:], E_sb[:],
                                         src[:, g * GS:(g + 1) * GS],
                                         start=True, stop=True)
                        Ps.append(P)
                    for g in range(G):
                        nc.vector.tensor_mul(dst[:, g * GS:(g + 1) * GS],
                                             ex[:, i * BC + g * GS:
                                                i * BC + (g + 1) * GS],
                                             Ps[g][:])
                    # Renorm trigger: tau = t = K(m+1); normalizer =
                    # P_tau[0, :] (any per-column scale works; the host uses
                    # the recorded fp16 reciprocal exactly).  Broadcast it on
                    # GPSIMD and fold into the ex slice of step tau+D, so the
                    # renorm never touches the serial matmul/multiply chain.
                    if t % K == 0 and t > 0 and t + D <= T - 1:
                        m = t // K - 1
                        for g in range(G):
                            nc.vector.reciprocal(
                                rh_sb[0:1, m * BC + g * GS:m * BC + (g + 1) * GS],
                                Ps[g][0:1, :])
                        Rbc = rbc_pool.tile([L, BC], DT)
                        nc.gpsimd.partition_broadcast(
                            Rbc[:], rh_sb[0:1, m * BC:(m + 1) * BC])
                        pend_R = (Rbc, t + D)
                # Capture row 127 of the finished ring (w_{16j+1..16j+16});
                # the double ring gives this DMA 16 steps of WAR slack.
                ring = rings[j % 2]
                nc.sync.dma_start(
                    hist[0:1, j * CAPB * BC:(j + 1) * CAPB * BC],
                    ring[127:128, :])
                if j - 2 in ex_tiles:
                    del ex_tiles[j - 2]

        # Final (E @ w_512)[127] for samples with len == T.
        Pf = ps_pool.tile([L, BC], F32, tag="P0")
        nc.tensor.matmul(Pf[:], E_sb[:], wslot(T), start=True, stop=True)
        capf = state_pool.tile([L, BC], DT)
        nc.vector.tensor_copy(capf[:], Pf[:])
        nc.sync.dma_start(hist[0:1, T * BC:T * BC + BC], capf[127:128, :])
        nc.sync.dma_start(rhist[0:1, :], rh_sb[:])

    nc.compile()
    return nc


def _get_nc():
    global _CACHED_NC
    if _CACHED_NC is None:
        _CACHED_NC = _build_bass()
    return _CACHED_NC


def run_on_device(x, transit_matrix, **spmd_kwargs):
    """Shard inputs, run the SPMD kernel on 8 cores, return per-core results."""
    xT = np.ascontiguousarray(np.asarray(x, np.float32).transpose(2, 1, 0))
    trT = np.ascontiguousarray(np.asarray(transit_matrix, np.float32).T)
    in_maps = []
    for c in range(NCORES):
        xc = np.ascontiguousarray(xT[:, :, c * BC:(c + 1) * BC]).reshape(L, T * BC)
        in_maps.append({"xT": xc, "trT": trT})
    nc = _get_nc()
    return bass_utils.run_bass_kernel_spmd(
        nc, in_maps, core_ids=list(range(NCORES)), **spmd_kwargs
    )


def finish_on_host(results, x, lengths):
    """Reconstruct alpha[b] in float64 from the device captures.

    fv_t = ln(w_t) + t*SCALE_LN + sum of ln(s_m) over norms applied before t
    (norm m: s_m = 1/r_m, r_m recorded; applied at step a_m = 4(m+1)+D).
    For len < T the capture is w_{len+1}[127] = exp(x[b,len,127])/256 *
    (E @ w_len)[127] (with the step-len renorm folded in when a_m == len), which
    collapses to the uniform formula below; for len == T the tail capture is
    (E @ w_512)[127] directly.
    """
    lengths = np.asarray(lengths).astype(np.int64)
    x = np.asarray(x)
    alpha = np.empty(B, np.float64)
    for c in range(NCORES):
        hist = results[c]["hist"].reshape(-1).astype(np.float64)
        rh = results[c]["rhist"].reshape(-1).astype(np.float64)
        lnS = -np.log(rh.reshape(NNORM, BC))          # ln s_m per norm m
        cum = np.zeros((NNORM + 1, BC))
        cum[1:] = np.cumsum(lnS, axis=0)
        hist_blk = hist[:T * BC].reshape(T, BC)       # hist_blk[t-1] = w_t[127]
        cap512 = hist[T * BC:]

        ln = lengths[c * BC:(c + 1) * BC]             # (BC,)
        bi = np.arange(BC)
        full = ln == T
        nf = ~full
        out = np.empty(BC, np.float64)
        out[full] = T * SCALE_LN + cum[NNORM, bi[full]] + np.log(cap512[full])
        cap = hist_blk[ln[nf], bi[nf]]                # w_{len+1}[127]
        x127 = x[c * BC + bi[nf], ln[nf], 127].astype(np.float64)
        # norms applied at a_m = K(m+1)+D <= len: count = (len-D)//K, clipped
        nidx = np.clip((ln[nf] - D) // K, 0, NNORM)
        out[nf] = (np.log(cap) - x127 + (ln[nf] + 1) * SCALE_LN
                   + cum[nidx, bi[nf]])
        alpha[c * BC:(c + 1) * BC] = out
    return alpha.astype(np.float32)


def _crf_alpha_single(xb, tr, length):
    """Exact single-sample CRF forward in float64 (rare-fallback path)."""
    NEG = -10000.0
    trd = np.asarray(tr, np.float64)
    fv = np.full(L, NEG)
    fv[0] = 0.0
    for t in range(int(length)):
        sc = trd + fv[None, :] + np.asarray(xb[t], np.float64)[:, None]
        m = sc.max(axis=1)
        fv = m + np.log(np.exp(sc - m[:, None]).sum(axis=1))
    term = fv + trd[L - 1]
    m = term.max()
    return m + np.log(np.exp(term - m).sum())


def kernel(x, transit_matrix, lengths):
    x = np.asarray(x, np.float32)
    assert x.shape == (B, T, L), x.shape
    res = run_on_device(x, transit_matrix)
    alpha = finish_on_host(res.results, x, lengths)
    # fp16 captures can in principle underflow to subnormal/zero for extreme
    # samples; recompute those few (if any) exactly on host.
    bad = ~np.isfinite(alpha)
    if bad.any():
        ln = np.asarray(lengths).astype(np.int64)
        for b in np.nonzero(bad)[0]:
            alpha[b] = _crf_alpha_single(x[b], transit_matrix, ln[b])
    return alpha
